# revision 9
# baseline (speedup 1.0000x reference)
"""Trainium2 Bass kernel for nn_FEDformerEncoder (8-core data parallel).

The reference network is, per layer:
    y  = mean_e( conv1d_same(x, w_e) + b_e )              (depthwise conv on W)
    q,k,v = y @ w{q,k,v}.T + b{q,k,v}                     ([rows, P])
    Q,K,V = fft(q),fft(k),fft(v)
    Wt = K * conj(Q) / sqrt(P) * V
    out = ifft(Wt).real @ wo.T + bo

Everything except the elementwise complex triple product is linear in x, so
the conv, the FFT, and the iFFT fold into host-precomputed projection
weights.  Real-input FFT symmetry packs each 1024-bin complex spectrum into
exactly 1024 reals: block A = Re[0..511], block B = [Re[512](Nyquist),
Im[1..511]].  Per layer the device work is then:

    S  = x @ EW + Sbias       # EW [2048, 3072] = packed q|k|v spectra proj
    Wt = complex-triple(S)    # elementwise, slot 0 of A/B special (DC/Nyq)
    x' = Wt @ WoP + bo        # WoP [1024, 2048] = packed inverse proj

Sharded batch-wise over 8 cores (4 batches = 512 rows per core), weights
replicated.  Activations live in SBUF in transposed layout [feature(part),
row(free)]; both matmul stages and the elementwise step all operate in this
layout so no on-device transposes are needed.  Matmuls run as float32r
(full-rate fp32 storage), accumulation in fp32 PSUM.
"""
import sys

import numpy as np

sys.path.insert(0, "/opt/trn_rl_repo")

import concourse.bass as bass
import concourse.mybir as mybir
import concourse.tile as tile
from concourse import bacc
from concourse.bass_utils import run_bass_kernel_spmd

BS, CNT, W, P, E, KK, L = 32, 128, 2048, 1024, 8, 25, 2
H = P // 2                    # 512 slots per packed block
NCORES = 8
ROWS = (BS // NCORES) * CNT   # 512 rows per core
KT = W // 128                 # 16 contraction tiles (forward)
MT = (3 * P) // 128           # 24 forward output tiles (q|k|v packed)
ST = P // 128                 # 8 contraction tiles (output matmul)
WT = W // 128                 # 16 output tiles
F32 = mybir.dt.float32
F32R = mybir.dt.float32r
IDENT = mybir.ActivationFunctionType.Identity


def _fold_layer(conv_w, conv_b, wq, bq, wk, bk, wv, bv, wo, bo):
    """Fold conv + FFT into projection weights (float64 math, fp32 out).

    Returns EW [W, 3*P], Sbias [3*P], WoP [P, W], bo [W].
    """
    f64 = np.float64
    wbar = conv_w.astype(f64).mean(axis=0)[0]          # [KK]
    bbar = conv_b.astype(f64).mean()

    idx = np.arange(W)
    D = idx[None, :] - idx[:, None] + (KK // 2)        # C[w,u] = wbar[u-w+12]
    C = np.where((D >= 0) & (D < KK), wbar[np.clip(D, 0, KK - 1)], 0.0)

    def pack_fwd(wmat, bvec, scale=1.0):
        Wf = np.fft.fft(wmat.astype(f64), axis=0)      # [P, W]
        Bf = np.fft.fft(bvec.astype(f64))              # [P]
        cols = np.empty((W, P), dtype=f64)
        cols[:, :H] = Wf[:H, :].real.T
        cols[:, H] = Wf[H, :].real
        cols[:, H + 1:] = Wf[1:H, :].imag.T
        bias = np.empty(P, dtype=f64)
        bias[:H] = Bf[:H].real
        bias[H] = Bf[H].real
        bias[H + 1:] = Bf[1:H].imag
        return cols * scale, bias * scale

    s = 1.0 / np.sqrt(f64(P))
    cq, bq_p = pack_fwd(wq, bq)
    ck, bk_p = pack_fwd(wk, bk)
    cv, bv_p = pack_fwd(wv, bv, scale=s)
    cols = np.concatenate([cq, ck, cv], axis=1)        # [W, 3P]
    bias = np.concatenate([bq_p, bk_p, bv_p])

    EW = C.T @ cols
    Sbias = bbar * cols.sum(axis=0) + bias

    G = np.fft.ifft(wo.astype(f64), axis=1)            # [W, P]
    WoP = np.empty((P, W), dtype=f64)
    WoP[0] = G[:, 0].real
    WoP[1:H] = 2.0 * G[:, 1:H].real.T
    WoP[H] = G[:, H].real
    WoP[H + 1:] = -2.0 * G[:, 1:H].imag.T

    return (EW.astype(np.float32), Sbias.astype(np.float32),
            WoP.astype(np.float32), bo.astype(np.float32))


def _build_module():
    nc = bacc.Bacc("TRN2", target_bir_lowering=False, debug=False)

    xin = nc.dram_tensor("xin", [128, KT * ROWS], F32R, kind="ExternalInput")
    wf = [nc.dram_tensor(f"wf{l}", [MT, 128, W], F32R, kind="ExternalInput")
          for l in range(L)]
    wo_ = [nc.dram_tensor(f"wo{l}", [WT, 128, P], F32R, kind="ExternalInput")
           for l in range(L)]
    bf = [nc.dram_tensor(f"bf{l}", [128, MT], F32, kind="ExternalInput")
          for l in range(L)]
    bo_ = [nc.dram_tensor(f"bo{l}", [128, WT], F32, kind="ExternalInput")
           for l in range(L)]
    xout = nc.dram_tensor("xout", [WT, 128, ROWS], F32, kind="ExternalOutput")

    with tile.TileContext(nc) as tc:
        with (
            tc.tile_pool(name="xbuf", bufs=1) as xpool,
            tc.tile_pool(name="bias", bufs=1) as bpool,
            tc.tile_pool(name="wfw", bufs=3) as wfpool,
            tc.tile_pool(name="wow", bufs=3) as wopool,
            tc.tile_pool(name="spec", bufs=MT) as spool,
            tc.tile_pool(name="wt", bufs=8) as wtpool,
            tc.tile_pool(name="ew", bufs=6) as ewpool,
            tc.tile_pool(name="psum", bufs=6, space="PSUM") as pspool,
        ):
            xA = xpool.tile([128, KT * ROWS], F32R, tag="xA")
            xB = xpool.tile([128, KT * ROWS], F32R, tag="xB")
            nc.sync.dma_start(xA[:], xin[:])

            bf_sb, bo_sb = [], []
            for l in range(L):
                bft = bpool.tile([128, MT], F32, tag=f"bf{l}")
                nc.sync.dma_start(bft[:], bf[l][:])
                bf_sb.append(bft)
                bot = bpool.tile([128, WT], F32, tag=f"bo{l}")
                nc.sync.dma_start(bot[:], bo_[l][:])
                bo_sb.append(bot)

            cur, nxt = xA, xB
            for l in range(L):
                # ---- forward: S = x @ EW + Sbias  (S^T layout) ----
                S = []
                for j in range(MT):
                    wtile = wfpool.tile([128, W], F32R, tag="wf")
                    nc.sync.dma_start(wtile[:], wf[l][j])
                    ps = pspool.tile([128, ROWS], F32, tag="ps")
                    for k in range(KT):
                        nc.tensor.matmul(
                            ps[:],
                            wtile[:, bass.ts(k, 128)],
                            cur[:, bass.ts(k, ROWS)],
                            start=(k == 0), stop=(k == KT - 1))
                    Sj = spool.tile([128, ROWS], F32, tag="spec")
                    nc.scalar.activation(Sj[:], ps[:], IDENT,
                                         bias=bf_sb[l][:, j:j + 1])
                    S.append(Sj)

                # ---- elementwise: Wt = K * conj(Q) * V(pre-scaled) ----
                WA, WB = [], []
                for t in range(4):
                    qA, qB = S[t], S[4 + t]
                    kA, kB = S[8 + t], S[12 + t]
                    vA, vB = S[16 + t], S[20 + t]
                    v = nc.vector
                    cr = ewpool.tile([128, ROWS], F32, tag="ew")
                    ci = ewpool.tile([128, ROWS], F32, tag="ew")
                    t0 = ewpool.tile([128, ROWS], F32, tag="ew")
                    v.tensor_mul(cr[:], kA[:], qA[:])
                    v.tensor_mul(t0[:], kB[:], qB[:])
                    v.tensor_add(cr[:], cr[:], t0[:])
                    v.tensor_mul(ci[:], kB[:], qA[:])
                    v.tensor_mul(t0[:], kA[:], qB[:])
                    v.tensor_sub(ci[:], ci[:], t0[:])
                    wr = wtpool.tile([128, ROWS], F32R, tag="wt")
                    wi = wtpool.tile([128, ROWS], F32R, tag="wt")
                    v.tensor_mul(wr[:], cr[:], vA[:])
                    v.tensor_mul(t0[:], ci[:], vB[:])
                    v.tensor_sub(wr[:], wr[:], t0[:])
                    v.tensor_mul(wi[:], cr[:], vB[:])
                    v.tensor_mul(t0[:], ci[:], vA[:])
                    v.tensor_add(wi[:], wi[:], t0[:])
                    if t == 0:
                        # slot 0: A holds DC, B holds Nyquist — both real
                        v.tensor_mul(t0[0:1, :], qA[0:1, :], kA[0:1, :])
                        v.tensor_mul(wr[0:1, :], t0[0:1, :], vA[0:1, :])
                        v.tensor_mul(t0[0:1, :], qB[0:1, :], kB[0:1, :])
                        v.tensor_mul(wi[0:1, :], t0[0:1, :], vB[0:1, :])
                    WA.append(wr)
                    WB.append(wi)
                Wcat = WA + WB

                # ---- output: x' = Wt @ WoP + bo  (back to x^T layout) ----
                for j in range(WT):
                    wotile = wopool.tile([128, P], F32R, tag="wo")
                    nc.sync.dma_start(wotile[:], wo_[l][j])
                    ps = pspool.tile([128, ROWS], F32, tag="ps")
                    for st in range(ST):
                        nc.tensor.matmul(
                            ps[:],
                            wotile[:, bass.ts(st, 128)],
                            Wcat[st][:],
                            start=(st == 0), stop=(st == ST - 1))
                    if l == L - 1:
                        # fp32 staging so the final output isn't f32r-rounded
                        ostage = ewpool.tile([128, ROWS], F32, tag="ew")
                        nc.vector.tensor_scalar_add(
                            ostage[:], ps[:], bo_sb[l][:, j:j + 1])
                        nc.sync.dma_start(xout[j], ostage[:])
                    else:
                        nc.vector.tensor_scalar_add(
                            nxt[:, bass.ts(j, ROWS)], ps[:],
                            bo_sb[l][:, j:j + 1])
                cur, nxt = nxt, cur
    nc.compile()
    return nc


_MODULE_CACHE = {}


def _get_module():
    if "nc" not in _MODULE_CACHE:
        _MODULE_CACHE["nc"] = _build_module()
    return _MODULE_CACHE["nc"]


def _prepare_weight_maps(conv_w, conv_b, wq, bq, wk, bk, wv, bv, wo, bo):
    m = {}
    for l in range(L):
        EW, Sbias, WoP, bol = _fold_layer(
            conv_w[l], conv_b[l], wq[l], bq[l], wk[l], bk[l],
            wv[l], bv[l], wo[l], bo[l])
        # lhsT tile layouts, partition-contiguous in DRAM
        m[f"wf{l}"] = np.ascontiguousarray(
            EW.reshape(KT, 128, MT, 128).transpose(2, 1, 0, 3)
            .reshape(MT, 128, W))
        m[f"wo{l}"] = np.ascontiguousarray(
            WoP.reshape(ST, 128, WT, 128).transpose(2, 1, 0, 3)
            .reshape(WT, 128, P))
        m[f"bf{l}"] = np.ascontiguousarray(Sbias.reshape(MT, 128).T)
        m[f"bo{l}"] = np.ascontiguousarray(bol.reshape(WT, 128).T)
    return m


def kernel(x, conv_w, conv_b, wq, bq, wk, bk, wv, bv, wo, bo):
    x = np.asarray(x, dtype=np.float32)
    wmap = _prepare_weight_maps(
        np.asarray(conv_w), np.asarray(conv_b),
        np.asarray(wq), np.asarray(bq), np.asarray(wk), np.asarray(bk),
        np.asarray(wv), np.asarray(bv), np.asarray(wo), np.asarray(bo))

    per_core = BS // NCORES
    in_maps = []
    for c in range(NCORES):
        xc = x[c * per_core:(c + 1) * per_core].reshape(ROWS, W)
        xin = np.ascontiguousarray(
            xc.reshape(ROWS, KT, 128).transpose(2, 1, 0).reshape(128, KT * ROWS))
        in_maps.append({"xin": xin, **wmap})

    nc = _get_module()
    res = run_bass_kernel_spmd(nc, in_maps, list(range(NCORES)))

    outs = []
    for c in range(NCORES):
        xo = res.results[c]["xout"]                    # [WT, 128, ROWS]
        outs.append(xo.transpose(2, 0, 1).reshape(per_core, CNT, W))
    return np.concatenate(outs, axis=0).astype(np.float32)


# revision 10
# speedup vs baseline: 1.2885x; 1.2885x over previous
"""Trainium2 Bass kernel for nn_FEDformerEncoder (8-core data parallel).

The reference network is, per layer:
    y  = mean_e( conv1d_same(x, w_e) + b_e )              (depthwise conv on W)
    q,k,v = y @ w{q,k,v}.T + b{q,k,v}                     ([rows, P])
    Q,K,V = fft(q),fft(k),fft(v)
    Wt = K * conj(Q) / sqrt(P) * V
    out = ifft(Wt).real @ wo.T + bo

Everything except the elementwise complex triple product is linear in x, so
the conv, the FFT, and the iFFT fold into host-precomputed projection
weights.  Real-input FFT symmetry packs each 1024-bin complex spectrum into
exactly 1024 reals: block A = Re[0..511], block B = [Re[512](Nyquist),
Im[1..511]].  Per layer the device work is then:

    S  = x @ EW + Sbias       # EW [2048, 3072] = packed q|k|v spectra proj
    Wt = complex-triple(S)    # elementwise, slot 0 of A/B special (DC/Nyq)
    x' = Wt @ WoP + bo        # WoP [1024, 2048] = packed inverse proj

Sharded batch-wise over 8 cores (4 batches = 512 rows per core), weights
replicated.  Activations live in SBUF in transposed layout [feature(part),
row(free)]; both matmul stages and the elementwise step operate in this
layout so no on-device transposes are needed.  Matmul operands are bf16
(fp32 PSUM accumulation, fp32 elementwise); the forward loop is grouped by
spectral partition-row t so the elementwise stage pipelines with matmuls,
and the output contraction consumes tiles in production order.
"""
import sys

import ml_dtypes
import numpy as np

sys.path.insert(0, "/opt/trn_rl_repo")

import concourse.bass as bass
import concourse.mybir as mybir
import concourse.tile as tile
from concourse import bacc
from concourse.bass_utils import run_bass_kernel_spmd

BS, CNT, W, P, E, KK, L = 32, 128, 2048, 1024, 8, 25, 2
H = P // 2                    # 512 slots per packed block
NCORES = 8
ROWS = (BS // NCORES) * CNT   # 512 rows per core
KT = W // 128                 # 16 contraction tiles (forward)
MT = (3 * P) // 128           # 24 forward output tiles (q|k|v packed)
ST = P // 128                 # 8 contraction tiles (output matmul)
WT = W // 128                 # 16 output tiles
F32 = mybir.dt.float32
BF16 = mybir.dt.bfloat16
IDENT = mybir.ActivationFunctionType.Identity
BF16_NP = ml_dtypes.bfloat16


def _fold_layer(conv_w, conv_b, wq, bq, wk, bk, wv, bv, wo, bo):
    """Fold conv + FFT into projection weights (float64 math).

    Returns EW [W, 3*P], Sbias [3*P], WoP [P, W] (rows interleaved
    [A0 B0 A1 B1 A2 B2 A3 B3] by 128-tile), bo [W].
    """
    f64 = np.float64
    wbar = conv_w.astype(f64).mean(axis=0)[0]          # [KK]
    bbar = conv_b.astype(f64).mean()

    idx = np.arange(W)
    D = idx[None, :] - idx[:, None] + (KK // 2)        # C[w,u] = wbar[u-w+12]
    C = np.where((D >= 0) & (D < KK), wbar[np.clip(D, 0, KK - 1)], 0.0)

    def pack_fwd(wmat, bvec, scale=1.0):
        Wf = np.fft.fft(wmat.astype(f64), axis=0)      # [P, W]
        Bf = np.fft.fft(bvec.astype(f64))              # [P]
        cols = np.empty((W, P), dtype=f64)
        cols[:, :H] = Wf[:H, :].real.T
        cols[:, H] = Wf[H, :].real
        cols[:, H + 1:] = Wf[1:H, :].imag.T
        bias = np.empty(P, dtype=f64)
        bias[:H] = Bf[:H].real
        bias[H] = Bf[H].real
        bias[H + 1:] = Bf[1:H].imag
        return cols * scale, bias * scale

    s = 1.0 / np.sqrt(f64(P))
    cq, bq_p = pack_fwd(wq, bq)
    ck, bk_p = pack_fwd(wk, bk)
    cv, bv_p = pack_fwd(wv, bv, scale=s)
    cols = np.concatenate([cq, ck, cv], axis=1)        # [W, 3P]
    bias = np.concatenate([bq_p, bk_p, bv_p])

    EW = C.T @ cols
    Sbias = bbar * cols.sum(axis=0) + bias

    G = np.fft.ifft(wo.astype(f64), axis=1)            # [W, P]
    WoP = np.empty((P, W), dtype=f64)
    WoP[0] = G[:, 0].real
    WoP[1:H] = 2.0 * G[:, 1:H].real.T
    WoP[H] = G[:, H].real
    WoP[H + 1:] = -2.0 * G[:, 1:H].imag.T
    # interleave row-tiles A0 B0 A1 B1 ... to match Wcat production order
    WoP = WoP.reshape(2, 4, 128, W).transpose(1, 0, 2, 3).reshape(P, W)

    return EW, Sbias.astype(np.float32), WoP, bo.astype(np.float32)


def _build_module():
    nc = bacc.Bacc("TRN2", target_bir_lowering=False, debug=False)

    xin = nc.dram_tensor("xin", [KT, 128, ROWS], BF16, kind="ExternalInput")
    wf = [nc.dram_tensor(f"wf{l}", [MT, 128, W], BF16, kind="ExternalInput")
          for l in range(L)]
    wo_ = [nc.dram_tensor(f"wo{l}", [WT, 128, P], BF16, kind="ExternalInput")
           for l in range(L)]
    bf = [nc.dram_tensor(f"bf{l}", [128, MT], F32, kind="ExternalInput")
          for l in range(L)]
    bo_ = [nc.dram_tensor(f"bo{l}", [128, WT], F32, kind="ExternalInput")
           for l in range(L)]
    xout = nc.dram_tensor("xout", [WT, 128, ROWS], F32, kind="ExternalOutput")

    with tile.TileContext(nc) as tc:
        with (
            tc.tile_pool(name="xbuf", bufs=2 * KT) as xpool,
            tc.tile_pool(name="bias", bufs=1) as bpool,
            tc.tile_pool(name="wfw", bufs=4) as wfpool,
            tc.tile_pool(name="wow", bufs=4) as wopool,
            tc.tile_pool(name="spec", bufs=14) as spool,
            tc.tile_pool(name="wt", bufs=8) as wtpool,
            tc.tile_pool(name="ew", bufs=6) as ewpool,
            tc.tile_pool(name="out", bufs=4) as opool,
            tc.tile_pool(name="psum", bufs=6, space="PSUM") as pspool,
        ):
            xcur = []
            for k in range(KT):
                xt = xpool.tile([128, ROWS], BF16, tag="x")
                nc.sync.dma_start(xt[:], xin[k])
                xcur.append(xt)

            bf_sb, bo_sb = [], []
            for l in range(L):
                bft = bpool.tile([128, MT], F32, tag=f"bf{l}")
                nc.sync.dma_start(bft[:], bf[l][:])
                bf_sb.append(bft)
                bot = bpool.tile([128, WT], F32, tag=f"bo{l}")
                nc.sync.dma_start(bot[:], bo_[l][:])
                bo_sb.append(bot)

            for l in range(L):
                # ---- forward + elementwise, pipelined over partition rows t
                # Wcat order: [WA0, WB0, WA1, WB1, ...] (matches WoP rows)
                Wcat = [None] * ST
                for t in range(4):
                    St = []
                    for b in range(6):       # qA qB kA kB vA vB row t
                        j = b * 4 + t
                        wtile = wfpool.tile([128, W], BF16, tag="wf")
                        nc.sync.dma_start(wtile[:], wf[l][j])
                        ps = pspool.tile([128, ROWS], F32, tag="ps")
                        for k in range(KT):
                            nc.tensor.matmul(
                                ps[:],
                                wtile[:, bass.ts(k, 128)],
                                xcur[k][:],
                                start=(k == 0), stop=(k == KT - 1))
                        Sj = spool.tile([128, ROWS], F32, tag="spec")
                        nc.scalar.activation(Sj[:], ps[:], IDENT,
                                             bias=bf_sb[l][:, j:j + 1])
                        St.append(Sj)
                    qA, qB, kA, kB, vA, vB = St
                    v = nc.vector
                    cr = ewpool.tile([128, ROWS], F32, tag="ew")
                    ci = ewpool.tile([128, ROWS], F32, tag="ew")
                    t0 = ewpool.tile([128, ROWS], F32, tag="ew")
                    v.tensor_mul(cr[:], kA[:], qA[:])
                    v.tensor_mul(t0[:], kB[:], qB[:])
                    v.tensor_add(cr[:], cr[:], t0[:])
                    v.tensor_mul(ci[:], kB[:], qA[:])
                    v.tensor_mul(t0[:], kA[:], qB[:])
                    v.tensor_sub(ci[:], ci[:], t0[:])
                    wr = wtpool.tile([128, ROWS], BF16, tag="wt")
                    wi = wtpool.tile([128, ROWS], BF16, tag="wt")
                    v.tensor_mul(wr[:], cr[:], vA[:])
                    v.tensor_mul(t0[:], ci[:], vB[:])
                    v.tensor_sub(wr[:], wr[:], t0[:])
                    v.tensor_mul(wi[:], cr[:], vB[:])
                    v.tensor_mul(t0[:], ci[:], vA[:])
                    v.tensor_add(wi[:], wi[:], t0[:])
                    if t == 0:
                        # slot 0: A holds DC, B holds Nyquist — both real
                        v.tensor_mul(t0[0:1, :], qA[0:1, :], kA[0:1, :])
                        v.tensor_mul(wr[0:1, :], t0[0:1, :], vA[0:1, :])
                        v.tensor_mul(t0[0:1, :], qB[0:1, :], kB[0:1, :])
                        v.tensor_mul(wi[0:1, :], t0[0:1, :], vB[0:1, :])
                    Wcat[2 * t] = wr
                    Wcat[2 * t + 1] = wi

                # ---- output: x' = Wt @ WoP + bo  (back to x^T layout) ----
                xnxt = []
                for j in range(WT):
                    wotile = wopool.tile([128, P], BF16, tag="wo")
                    nc.sync.dma_start(wotile[:], wo_[l][j])
                    ps = pspool.tile([128, ROWS], F32, tag="ps")
                    for st in range(ST):
                        nc.tensor.matmul(
                            ps[:],
                            wotile[:, bass.ts(st, 128)],
                            Wcat[st][:],
                            start=(st == 0), stop=(st == ST - 1))
                    if l == L - 1:
                        ostage = opool.tile([128, ROWS], F32, tag="out")
                        nc.vector.tensor_scalar_add(
                            ostage[:], ps[:], bo_sb[l][:, j:j + 1])
                        nc.sync.dma_start(xout[j], ostage[:])
                    else:
                        xt = xpool.tile([128, ROWS], BF16, tag="x")
                        nc.vector.tensor_scalar_add(
                            xt[:], ps[:], bo_sb[l][:, j:j + 1])
                        xnxt.append(xt)
                if l < L - 1:
                    xcur = xnxt
    nc.compile()
    return nc


_MODULE_CACHE = {}


def _get_module():
    if "nc" not in _MODULE_CACHE:
        _MODULE_CACHE["nc"] = _build_module()
    return _MODULE_CACHE["nc"]


def _prepare_weight_maps(conv_w, conv_b, wq, bq, wk, bk, wv, bv, wo, bo):
    m = {}
    for l in range(L):
        EW, Sbias, WoP, bol = _fold_layer(
            conv_w[l], conv_b[l], wq[l], bq[l], wk[l], bk[l],
            wv[l], bv[l], wo[l], bo[l])
        # lhsT tile layouts, partition-contiguous in DRAM
        m[f"wf{l}"] = np.ascontiguousarray(
            EW.reshape(KT, 128, MT, 128).transpose(2, 1, 0, 3)
            .reshape(MT, 128, W).astype(BF16_NP))
        m[f"wo{l}"] = np.ascontiguousarray(
            WoP.reshape(ST, 128, WT, 128).transpose(2, 1, 0, 3)
            .reshape(WT, 128, P).astype(BF16_NP))
        m[f"bf{l}"] = np.ascontiguousarray(Sbias.reshape(MT, 128).T)
        m[f"bo{l}"] = np.ascontiguousarray(bol.reshape(WT, 128).T)
    return m


def kernel(x, conv_w, conv_b, wq, bq, wk, bk, wv, bv, wo, bo):
    x = np.asarray(x, dtype=np.float32)
    wmap = _prepare_weight_maps(
        np.asarray(conv_w), np.asarray(conv_b),
        np.asarray(wq), np.asarray(bq), np.asarray(wk), np.asarray(bk),
        np.asarray(wv), np.asarray(bv), np.asarray(wo), np.asarray(bo))

    per_core = BS // NCORES
    in_maps = []
    for c in range(NCORES):
        xc = x[c * per_core:(c + 1) * per_core].reshape(ROWS, W)
        xin = np.ascontiguousarray(
            xc.reshape(ROWS, KT, 128).transpose(1, 2, 0).astype(BF16_NP))
        in_maps.append({"xin": xin, **wmap})

    nc = _get_module()
    res = run_bass_kernel_spmd(nc, in_maps, list(range(NCORES)))

    outs = []
    for c in range(NCORES):
        xo = res.results[c]["xout"]                    # [WT, 128, ROWS]
        outs.append(xo.transpose(2, 0, 1).reshape(per_core, CNT, W))
    return np.concatenate(outs, axis=0).astype(np.float32)


# revision 11
# speedup vs baseline: 1.3007x; 1.0095x over previous
"""Trainium2 Bass kernel for nn_FEDformerEncoder (8-core data parallel).

The reference network is, per layer:
    y  = mean_e( conv1d_same(x, w_e) + b_e )              (depthwise conv on W)
    q,k,v = y @ w{q,k,v}.T + b{q,k,v}                     ([rows, P])
    Q,K,V = fft(q),fft(k),fft(v)
    Wt = K * conj(Q) / sqrt(P) * V
    out = ifft(Wt).real @ wo.T + bo

Everything except the elementwise complex triple product is linear in x, so
the conv, the FFT, and the iFFT fold into host-precomputed projection
weights.  Real-input FFT symmetry packs each 1024-bin complex spectrum into
exactly 1024 reals: block A = Re[0..511], block B = [Re[512](Nyquist),
Im[1..511]].  Per layer the device work is then:

    S  = x @ EW + Sbias       # EW [2048, 3072] = packed q|k|v spectra proj
    Wt = complex-triple(S)    # elementwise, slot 0 of A/B special (DC/Nyq)
    x' = Wt @ WoP + bo        # WoP [1024, 2048] = packed inverse proj

Sharded batch-wise over 8 cores (4 batches = 512 rows per core), weights
replicated.  Activations live in SBUF in transposed layout [feature(part),
row(free)]; both matmul stages and the elementwise step operate in this
layout so no on-device transposes are needed.  Matmul operands are bf16
(fp32 PSUM accumulation, fp32 elementwise); the forward loop is grouped by
spectral partition-row t so the elementwise stage pipelines with matmuls,
and the output contraction consumes tiles in production order.
"""
import sys

import numpy as np

sys.path.insert(0, "/opt/trn_rl_repo")

import concourse.bass as bass
import concourse.mybir as mybir
import concourse.tile as tile
from concourse import bacc
from concourse.bass_utils import run_bass_kernel_spmd

BS, CNT, W, P, E, KK, L = 32, 128, 2048, 1024, 8, 25, 2
H = P // 2                    # 512 slots per packed block
NCORES = 8
ROWS = (BS // NCORES) * CNT   # 512 rows per core
KT = W // 128                 # 16 contraction tiles (forward)
MT = (3 * P) // 128           # 24 forward output tiles (q|k|v packed)
ST = P // 128                 # 8 contraction tiles (output matmul)
WT = W // 128                 # 16 output tiles
F32 = mybir.dt.float32
ACT = mybir.dt.float16
IDENT = mybir.ActivationFunctionType.Identity
ACT_NP = np.float16


def _fold_layer(conv_w, conv_b, wq, bq, wk, bk, wv, bv, wo, bo):
    """Fold conv + FFT into projection weights (float64 math).

    Returns EW [W, 3*P], Sbias [3*P], WoP [P, W] (rows interleaved
    [A0 B0 A1 B1 A2 B2 A3 B3] by 128-tile), bo [W].
    """
    f64 = np.float64
    wbar = conv_w.astype(f64).mean(axis=0)[0]          # [KK]
    bbar = conv_b.astype(f64).mean()

    idx = np.arange(W)
    D = idx[None, :] - idx[:, None] + (KK // 2)        # C[w,u] = wbar[u-w+12]
    C = np.where((D >= 0) & (D < KK), wbar[np.clip(D, 0, KK - 1)], 0.0)

    def pack_fwd(wmat, bvec, scale=1.0):
        Wf = np.fft.fft(wmat.astype(f64), axis=0)      # [P, W]
        Bf = np.fft.fft(bvec.astype(f64))              # [P]
        cols = np.empty((W, P), dtype=f64)
        cols[:, :H] = Wf[:H, :].real.T
        cols[:, H] = Wf[H, :].real
        cols[:, H + 1:] = Wf[1:H, :].imag.T
        bias = np.empty(P, dtype=f64)
        bias[:H] = Bf[:H].real
        bias[H] = Bf[H].real
        bias[H + 1:] = Bf[1:H].imag
        return cols * scale, bias * scale

    s = 1.0 / np.sqrt(f64(P))
    cq, bq_p = pack_fwd(wq, bq)
    ck, bk_p = pack_fwd(wk, bk)
    cv, bv_p = pack_fwd(wv, bv, scale=s)
    cols = np.concatenate([cq, ck, cv], axis=1)        # [W, 3P]
    bias = np.concatenate([bq_p, bk_p, bv_p])

    EW = C.T @ cols
    Sbias = bbar * cols.sum(axis=0) + bias

    G = np.fft.ifft(wo.astype(f64), axis=1)            # [W, P]
    WoP = np.empty((P, W), dtype=f64)
    WoP[0] = G[:, 0].real
    WoP[1:H] = 2.0 * G[:, 1:H].real.T
    WoP[H] = G[:, H].real
    WoP[H + 1:] = -2.0 * G[:, 1:H].imag.T
    # interleave row-tiles A0 B0 A1 B1 ... to match Wcat production order
    WoP = WoP.reshape(2, 4, 128, W).transpose(1, 0, 2, 3).reshape(P, W)

    return EW, Sbias.astype(np.float32), WoP, bo.astype(np.float32)


def _build_module():
    nc = bacc.Bacc("TRN2", target_bir_lowering=False, debug=False)

    xin = nc.dram_tensor("xin", [KT, 128, ROWS], ACT, kind="ExternalInput")
    wf = [nc.dram_tensor(f"wf{l}", [MT, 128, W], ACT, kind="ExternalInput")
          for l in range(L)]
    wo_ = [nc.dram_tensor(f"wo{l}", [WT, 128, P], ACT, kind="ExternalInput")
           for l in range(L)]
    bf = [nc.dram_tensor(f"bf{l}", [128, MT], F32, kind="ExternalInput")
          for l in range(L)]
    bo_ = [nc.dram_tensor(f"bo{l}", [128, WT], F32, kind="ExternalInput")
           for l in range(L)]
    xout = nc.dram_tensor("xout", [WT, 128, ROWS], F32, kind="ExternalOutput")

    with tile.TileContext(nc) as tc:
        with (
            tc.tile_pool(name="xbuf", bufs=2 * KT) as xpool,
            tc.tile_pool(name="bias", bufs=1) as bpool,
            tc.tile_pool(name="wfw", bufs=4) as wfpool,
            tc.tile_pool(name="wow", bufs=4) as wopool,
            tc.tile_pool(name="spec", bufs=14) as spool,
            tc.tile_pool(name="wt", bufs=8) as wtpool,
            tc.tile_pool(name="ew", bufs=6) as ewpool,
            tc.tile_pool(name="out", bufs=4) as opool,
            tc.tile_pool(name="psum", bufs=6, space="PSUM") as pspool,
        ):
            xcur = []
            for k in range(KT):
                xt = xpool.tile([128, ROWS], ACT, tag="x")
                nc.sync.dma_start(xt[:], xin[k])
                xcur.append(xt)

            bf_sb, bo_sb = [], []
            for l in range(L):
                bft = bpool.tile([128, MT], F32, tag=f"bf{l}")
                nc.sync.dma_start(bft[:], bf[l][:])
                bf_sb.append(bft)
                bot = bpool.tile([128, WT], F32, tag=f"bo{l}")
                nc.sync.dma_start(bot[:], bo_[l][:])
                bo_sb.append(bot)

            for l in range(L):
                # ---- forward + elementwise, pipelined over partition rows t
                # Wcat order: [WA0, WB0, WA1, WB1, ...] (matches WoP rows)
                Wcat = [None] * ST
                for t in range(4):
                    St = []
                    for b in range(6):       # qA qB kA kB vA vB row t
                        j = b * 4 + t
                        wtile = wfpool.tile([128, W], ACT, tag="wf")
                        nc.sync.dma_start(wtile[:], wf[l][j])
                        ps = pspool.tile([128, ROWS], F32, tag="ps")
                        for k in range(KT):
                            nc.tensor.matmul(
                                ps[:],
                                wtile[:, bass.ts(k, 128)],
                                xcur[k][:],
                                start=(k == 0), stop=(k == KT - 1))
                        Sj = spool.tile([128, ROWS], F32, tag="spec")
                        nc.scalar.activation(Sj[:], ps[:], IDENT,
                                             bias=bf_sb[l][:, j:j + 1])
                        St.append(Sj)
                    qA, qB, kA, kB, vA, vB = St
                    v = nc.vector
                    cr = ewpool.tile([128, ROWS], F32, tag="ew")
                    ci = ewpool.tile([128, ROWS], F32, tag="ew")
                    t0 = ewpool.tile([128, ROWS], F32, tag="ew")
                    v.tensor_mul(cr[:], kA[:], qA[:])
                    v.tensor_mul(t0[:], kB[:], qB[:])
                    v.tensor_add(cr[:], cr[:], t0[:])
                    v.tensor_mul(ci[:], kB[:], qA[:])
                    v.tensor_mul(t0[:], kA[:], qB[:])
                    v.tensor_sub(ci[:], ci[:], t0[:])
                    wr = wtpool.tile([128, ROWS], ACT, tag="wt")
                    wi = wtpool.tile([128, ROWS], ACT, tag="wt")
                    v.tensor_mul(wr[:], cr[:], vA[:])
                    v.tensor_mul(t0[:], ci[:], vB[:])
                    v.tensor_sub(wr[:], wr[:], t0[:])
                    v.tensor_mul(wi[:], cr[:], vB[:])
                    v.tensor_mul(t0[:], ci[:], vA[:])
                    v.tensor_add(wi[:], wi[:], t0[:])
                    if t == 0:
                        # slot 0: A holds DC, B holds Nyquist — both real
                        v.tensor_mul(t0[0:1, :], qA[0:1, :], kA[0:1, :])
                        v.tensor_mul(wr[0:1, :], t0[0:1, :], vA[0:1, :])
                        v.tensor_mul(t0[0:1, :], qB[0:1, :], kB[0:1, :])
                        v.tensor_mul(wi[0:1, :], t0[0:1, :], vB[0:1, :])
                    Wcat[2 * t] = wr
                    Wcat[2 * t + 1] = wi

                # ---- output: x' = Wt @ WoP + bo  (back to x^T layout) ----
                xnxt = []
                for j in range(WT):
                    wotile = wopool.tile([128, P], ACT, tag="wo")
                    nc.sync.dma_start(wotile[:], wo_[l][j])
                    ps = pspool.tile([128, ROWS], F32, tag="ps")
                    for st in range(ST):
                        nc.tensor.matmul(
                            ps[:],
                            wotile[:, bass.ts(st, 128)],
                            Wcat[st][:],
                            start=(st == 0), stop=(st == ST - 1))
                    if l == L - 1:
                        ostage = opool.tile([128, ROWS], F32, tag="out")
                        nc.vector.tensor_scalar_add(
                            ostage[:], ps[:], bo_sb[l][:, j:j + 1])
                        nc.sync.dma_start(xout[j], ostage[:])
                    else:
                        xt = xpool.tile([128, ROWS], ACT, tag="x")
                        nc.vector.tensor_scalar_add(
                            xt[:], ps[:], bo_sb[l][:, j:j + 1])
                        xnxt.append(xt)
                if l < L - 1:
                    xcur = xnxt
    nc.compile()
    return nc


_MODULE_CACHE = {}


def _get_module():
    if "nc" not in _MODULE_CACHE:
        _MODULE_CACHE["nc"] = _build_module()
    return _MODULE_CACHE["nc"]


def _prepare_weight_maps(conv_w, conv_b, wq, bq, wk, bk, wv, bv, wo, bo):
    m = {}
    for l in range(L):
        EW, Sbias, WoP, bol = _fold_layer(
            conv_w[l], conv_b[l], wq[l], bq[l], wk[l], bk[l],
            wv[l], bv[l], wo[l], bo[l])
        # lhsT tile layouts, partition-contiguous in DRAM
        m[f"wf{l}"] = np.ascontiguousarray(
            EW.reshape(KT, 128, MT, 128).transpose(2, 1, 0, 3)
            .reshape(MT, 128, W).astype(ACT_NP))
        m[f"wo{l}"] = np.ascontiguousarray(
            WoP.reshape(ST, 128, WT, 128).transpose(2, 1, 0, 3)
            .reshape(WT, 128, P).astype(ACT_NP))
        m[f"bf{l}"] = np.ascontiguousarray(Sbias.reshape(MT, 128).T)
        m[f"bo{l}"] = np.ascontiguousarray(bol.reshape(WT, 128).T)
    return m


def kernel(x, conv_w, conv_b, wq, bq, wk, bk, wv, bv, wo, bo):
    x = np.asarray(x, dtype=np.float32)
    wmap = _prepare_weight_maps(
        np.asarray(conv_w), np.asarray(conv_b),
        np.asarray(wq), np.asarray(bq), np.asarray(wk), np.asarray(bk),
        np.asarray(wv), np.asarray(bv), np.asarray(wo), np.asarray(bo))

    per_core = BS // NCORES
    in_maps = []
    for c in range(NCORES):
        xc = x[c * per_core:(c + 1) * per_core].reshape(ROWS, W)
        xin = np.ascontiguousarray(
            xc.reshape(ROWS, KT, 128).transpose(1, 2, 0).astype(ACT_NP))
        in_maps.append({"xin": xin, **wmap})

    nc = _get_module()
    res = run_bass_kernel_spmd(nc, in_maps, list(range(NCORES)))

    outs = []
    for c in range(NCORES):
        xo = res.results[c]["xout"]                    # [WT, 128, ROWS]
        outs.append(xo.transpose(2, 0, 1).reshape(per_core, CNT, W))
    return np.concatenate(outs, axis=0).astype(np.float32)


# revision 12
# speedup vs baseline: 1.8004x; 1.3843x over previous
"""Trainium2 Bass kernel for nn_FEDformerEncoder (8-core data parallel).

The reference network is, per layer (L=2):
    y  = mean_e( conv1d_same(x, w_e) + b_e )              (depthwise conv on W)
    q,k,v = y @ w{q,k,v}.T + b{q,k,v}                     ([rows, P])
    Q,K,V = fft(q),fft(k),fft(v)
    Wt = K * conj(Q) / sqrt(P) * V
    out = ifft(Wt).real @ wo.T + bo

Everything except the elementwise complex triple product is linear in x, so
the conv, the FFT, and the iFFT fold into host-precomputed projection
weights.  Real-input FFT symmetry packs each 1024-bin complex spectrum into
exactly 1024 reals per signal: block A = Re[0..511], block B =
[Re[512](Nyquist), Im[1..511]].  Composing the two layers' linear maps
(iFFT-projection of layer 1 directly into conv+FFT-projection of layer 2)
collapses the whole network into three matmul stages and two elementwise
stages:

    S1  = x   @ EW1  + b1     # [rows,2048] @ [2048,3072]
    Wt1 = complex-triple(S1)  # packed; slot 0 of A/B = DC/Nyquist, real
    S2  = Wt1 @ M12  + b2     # [rows,1024] @ [1024,3072], M12 = WoP1@EW2
    Wt2 = complex-triple(S2)
    out = Wt2 @ WoP2 + bo2    # [rows,1024] @ [1024,2048]

Sharded batch-wise over 8 cores (4 batches = 512 rows per core), weights
replicated.  Activations live in SBUF in transposed layout [feature(part),
row(free)] throughout, so no on-device transposes are needed.  Matmul
operands are fp16 (fp32 PSUM accumulation, fp32 elementwise); contraction
row-tiles are interleaved [A0 B0 A1 B1 ...] so each elementwise group
feeds the next stage in production order and the stages pipeline.
"""
import sys

import numpy as np

sys.path.insert(0, "/opt/trn_rl_repo")

import concourse.bass as bass
import concourse.mybir as mybir
import concourse.tile as tile
from concourse import bacc
from concourse.bass_utils import run_bass_kernel_spmd

BS, CNT, W, P, E, KK, L = 32, 128, 2048, 1024, 8, 25, 2
H = P // 2                    # 512 slots per packed block
NCORES = 8
ROWS = (BS // NCORES) * CNT   # 512 rows per core
KT = W // 128                 # 16 contraction tiles (stage 1)
MT = (3 * P) // 128           # 24 output tiles (stages 1,2: q|k|v packed)
ST = P // 128                 # 8 contraction tiles (stages 2,3)
WT = W // 128                 # 16 output tiles (stage 3)
F32 = mybir.dt.float32
ACT = mybir.dt.float16
ACT_NP = np.float16
IDENT = mybir.ActivationFunctionType.Identity


def _fold_layer(conv_w, conv_b, wq, bq, wk, bk, wv, bv, wo, bo):
    """Fold conv + FFT into projection weights (float64 math).

    Returns EW [W, 3*P], Sbias [3*P], WoP [P, W] (rows interleaved
    [A0 B0 A1 B1 A2 B2 A3 B3] by 128-tile), bo [W].
    """
    f64 = np.float64
    wbar = conv_w.astype(f64).mean(axis=0)[0]          # [KK]
    bbar = conv_b.astype(f64).mean()

    idx = np.arange(W)
    D = idx[None, :] - idx[:, None] + (KK // 2)        # C[w,u] = wbar[u-w+12]
    C = np.where((D >= 0) & (D < KK), wbar[np.clip(D, 0, KK - 1)], 0.0)

    def pack_fwd(wmat, bvec, scale=1.0):
        Wf = np.fft.fft(wmat.astype(f64), axis=0)      # [P, W]
        Bf = np.fft.fft(bvec.astype(f64))              # [P]
        cols = np.empty((W, P), dtype=f64)
        cols[:, :H] = Wf[:H, :].real.T
        cols[:, H] = Wf[H, :].real
        cols[:, H + 1:] = Wf[1:H, :].imag.T
        bias = np.empty(P, dtype=f64)
        bias[:H] = Bf[:H].real
        bias[H] = Bf[H].real
        bias[H + 1:] = Bf[1:H].imag
        return cols * scale, bias * scale

    s = 1.0 / np.sqrt(f64(P))
    cq, bq_p = pack_fwd(wq, bq)
    ck, bk_p = pack_fwd(wk, bk)
    cv, bv_p = pack_fwd(wv, bv, scale=s)
    cols = np.concatenate([cq, ck, cv], axis=1)        # [W, 3P]
    bias = np.concatenate([bq_p, bk_p, bv_p])

    EW = C.T @ cols
    Sbias = bbar * cols.sum(axis=0) + bias

    G = np.fft.ifft(wo.astype(f64), axis=1)            # [W, P]
    WoP = np.empty((P, W), dtype=f64)
    WoP[0] = G[:, 0].real
    WoP[1:H] = 2.0 * G[:, 1:H].real.T
    WoP[H] = G[:, H].real
    WoP[H + 1:] = -2.0 * G[:, 1:H].imag.T
    # interleave row-tiles A0 B0 A1 B1 ... to match Wcat production order
    WoP = WoP.reshape(2, 4, 128, W).transpose(1, 0, 2, 3).reshape(P, W)

    return EW, Sbias, WoP, bo.astype(f64)


def _build_module():
    nc = bacc.Bacc("TRN2", target_bir_lowering=False, debug=False)

    xin = nc.dram_tensor("xin", [KT, 128, ROWS], ACT, kind="ExternalInput")
    ew1 = nc.dram_tensor("ew1", [MT, 128, W], ACT, kind="ExternalInput")
    m12 = nc.dram_tensor("m12", [MT, 128, P], ACT, kind="ExternalInput")
    wop2 = nc.dram_tensor("wop2", [WT, 128, P], ACT, kind="ExternalInput")
    sb1 = nc.dram_tensor("sb1", [128, MT], F32, kind="ExternalInput")
    sb2 = nc.dram_tensor("sb2", [128, MT], F32, kind="ExternalInput")
    bo2 = nc.dram_tensor("bo2", [128, WT], F32, kind="ExternalInput")
    xout = nc.dram_tensor("xout", [WT, 128, ROWS], F32, kind="ExternalOutput")

    with tile.TileContext(nc) as tc:
        with (
            tc.tile_pool(name="xbuf", bufs=KT) as xpool,
            tc.tile_pool(name="bias", bufs=1) as bpool,
            tc.tile_pool(name="wfw", bufs=4) as wfpool,
            tc.tile_pool(name="wm", bufs=6) as wmpool,
            tc.tile_pool(name="spec", bufs=14) as spool,
            tc.tile_pool(name="wt", bufs=16) as wtpool,
            tc.tile_pool(name="ew", bufs=6) as ewpool,
            tc.tile_pool(name="out", bufs=4) as opool,
            tc.tile_pool(name="psum", bufs=6, space="PSUM") as pspool,
        ):
            xcur = []
            for k in range(KT):
                xt = xpool.tile([128, ROWS], ACT, tag="x")
                nc.sync.dma_start(xt[:], xin[k])
                xcur.append(xt)

            sb1_t = bpool.tile([128, MT], F32, tag="sb1")
            nc.sync.dma_start(sb1_t[:], sb1[:])
            sb2_t = bpool.tile([128, MT], F32, tag="sb2")
            nc.sync.dma_start(sb2_t[:], sb2[:])
            bo2_t = bpool.tile([128, WT], F32, tag="bo2")
            nc.sync.dma_start(bo2_t[:], bo2[:])

            def elementwise(St, first):
                """complex triple product on one partition-row group."""
                qA, qB, kA, kB, vA, vB = St
                v = nc.vector
                cr = ewpool.tile([128, ROWS], F32, tag="ew")
                ci = ewpool.tile([128, ROWS], F32, tag="ew")
                t0 = ewpool.tile([128, ROWS], F32, tag="ew")
                v.tensor_mul(cr[:], kA[:], qA[:])
                v.tensor_mul(t0[:], kB[:], qB[:])
                v.tensor_add(cr[:], cr[:], t0[:])
                v.tensor_mul(ci[:], kB[:], qA[:])
                v.tensor_mul(t0[:], kA[:], qB[:])
                v.tensor_sub(ci[:], ci[:], t0[:])
                wr = wtpool.tile([128, ROWS], ACT, tag="wt")
                wi = wtpool.tile([128, ROWS], ACT, tag="wt")
                v.tensor_mul(wr[:], cr[:], vA[:])
                v.tensor_mul(t0[:], ci[:], vB[:])
                v.tensor_sub(wr[:], wr[:], t0[:])
                v.tensor_mul(wi[:], cr[:], vB[:])
                v.tensor_mul(t0[:], ci[:], vA[:])
                v.tensor_add(wi[:], wi[:], t0[:])
                if first:
                    # slot 0: A holds DC, B holds Nyquist — both real
                    v.tensor_mul(t0[0:1, :], qA[0:1, :], kA[0:1, :])
                    v.tensor_mul(wr[0:1, :], t0[0:1, :], vA[0:1, :])
                    v.tensor_mul(t0[0:1, :], qB[0:1, :], kB[0:1, :])
                    v.tensor_mul(wi[0:1, :], t0[0:1, :], vB[0:1, :])
                return wr, wi

            # ---- stage 1: S1 = x @ EW1 + b1, pipelined elementwise ----
            Wcat1 = [None] * ST
            for t in range(4):
                St = []
                for b in range(6):           # qA qB kA kB vA vB row t
                    j = b * 4 + t
                    wtile = wfpool.tile([128, W], ACT, tag="wf")
                    nc.sync.dma_start(wtile[:], ew1[j])
                    ps = pspool.tile([128, ROWS], F32, tag="ps")
                    for k in range(KT):
                        nc.tensor.matmul(
                            ps[:], wtile[:, bass.ts(k, 128)], xcur[k][:],
                            start=(k == 0), stop=(k == KT - 1))
                    Sj = spool.tile([128, ROWS], F32, tag="spec")
                    nc.scalar.activation(Sj[:], ps[:], IDENT,
                                         bias=sb1_t[:, j:j + 1])
                    St.append(Sj)
                wr, wi = elementwise(St, t == 0)
                Wcat1[2 * t] = wr
                Wcat1[2 * t + 1] = wi

            # ---- stage 2: S2 = Wt1 @ M12 + b2, pipelined elementwise ----
            Wcat2 = [None] * ST
            for t in range(4):
                St = []
                for b in range(6):
                    j = b * 4 + t
                    wtile = wmpool.tile([128, P], ACT, tag="wm")
                    nc.sync.dma_start(wtile[:], m12[j])
                    ps = pspool.tile([128, ROWS], F32, tag="ps")
                    for s in range(ST):
                        nc.tensor.matmul(
                            ps[:], wtile[:, bass.ts(s, 128)], Wcat1[s][:],
                            start=(s == 0), stop=(s == ST - 1))
                    Sj = spool.tile([128, ROWS], F32, tag="spec")
                    nc.scalar.activation(Sj[:], ps[:], IDENT,
                                         bias=sb2_t[:, j:j + 1])
                    St.append(Sj)
                wr, wi = elementwise(St, t == 0)
                Wcat2[2 * t] = wr
                Wcat2[2 * t + 1] = wi

            # ---- stage 3: out = Wt2 @ WoP2 + bo2 ----
            for j in range(WT):
                wtile = wmpool.tile([128, P], ACT, tag="wm")
                nc.sync.dma_start(wtile[:], wop2[j])
                ps = pspool.tile([128, ROWS], F32, tag="ps")
                for s in range(ST):
                    nc.tensor.matmul(
                        ps[:], wtile[:, bass.ts(s, 128)], Wcat2[s][:],
                        start=(s == 0), stop=(s == ST - 1))
                ostage = opool.tile([128, ROWS], F32, tag="out")
                nc.vector.tensor_scalar_add(ostage[:], ps[:],
                                            bo2_t[:, j:j + 1])
                nc.sync.dma_start(xout[j], ostage[:])
    nc.compile()
    return nc


_MODULE_CACHE = {}


def _get_module():
    if "nc" not in _MODULE_CACHE:
        _MODULE_CACHE["nc"] = _build_module()
    return _MODULE_CACHE["nc"]


def _prepare_weight_maps(conv_w, conv_b, wq, bq, wk, bk, wv, bv, wo, bo):
    folds = [_fold_layer(conv_w[l], conv_b[l], wq[l], bq[l], wk[l], bk[l],
                         wv[l], bv[l], wo[l], bo[l]) for l in range(L)]
    EW1, Sb1, WoP1, _bo1 = folds[0]
    EW2, Sb2, WoP2, bo2 = folds[1]
    M12 = WoP1 @ EW2                               # [P, 3P], fp64
    Sb2e = _bo1 @ EW2 + Sb2                        # [3P]

    def pack(Wm, n_k, n_m):
        # [n_k*128, n_m*128] -> [n_m, 128, n_k*128] partition-contiguous
        return np.ascontiguousarray(
            Wm.reshape(n_k, 128, n_m, 128).transpose(2, 1, 0, 3)
            .reshape(n_m, 128, n_k * 128).astype(ACT_NP))

    return {
        "ew1": pack(EW1, KT, MT),
        "m12": pack(M12, ST, MT),
        "wop2": pack(WoP2, ST, WT),
        "sb1": np.ascontiguousarray(
            Sb1.reshape(MT, 128).T.astype(np.float32)),
        "sb2": np.ascontiguousarray(
            Sb2e.reshape(MT, 128).T.astype(np.float32)),
        "bo2": np.ascontiguousarray(
            bo2.reshape(WT, 128).T.astype(np.float32)),
    }


def _make_in_maps(inputs):
    x = np.asarray(inputs["x"], dtype=np.float32)
    wmap = _prepare_weight_maps(
        np.asarray(inputs["conv_w"]), np.asarray(inputs["conv_b"]),
        np.asarray(inputs["wq"]), np.asarray(inputs["bq"]),
        np.asarray(inputs["wk"]), np.asarray(inputs["bk"]),
        np.asarray(inputs["wv"]), np.asarray(inputs["bv"]),
        np.asarray(inputs["wo"]), np.asarray(inputs["bo"]))
    per_core = BS // NCORES
    in_maps = []
    for c in range(NCORES):
        xc = x[c * per_core:(c + 1) * per_core].reshape(ROWS, W)
        xin = np.ascontiguousarray(
            xc.reshape(ROWS, KT, 128).transpose(1, 2, 0).astype(ACT_NP))
        in_maps.append({"xin": xin, **wmap})
    return in_maps


def kernel(x, conv_w, conv_b, wq, bq, wk, bk, wv, bv, wo, bo):
    in_maps = _make_in_maps(dict(
        x=x, conv_w=conv_w, conv_b=conv_b, wq=wq, bq=bq, wk=wk, bk=bk,
        wv=wv, bv=bv, wo=wo, bo=bo))
    nc = _get_module()
    res = run_bass_kernel_spmd(nc, in_maps, list(range(NCORES)))

    per_core = BS // NCORES
    outs = []
    for c in range(NCORES):
        xo = res.results[c]["xout"]                    # [WT, 128, ROWS]
        outs.append(xo.transpose(2, 0, 1).reshape(per_core, CNT, W))
    return np.concatenate(outs, axis=0).astype(np.float32)
